# revision 57
# baseline (speedup 1.0000x reference)
"""Dense GAT layer (attention + out-proj + residual + LayerNorm + SiLU + node mask)
as a fused Bass/Tile kernel on 8 Trainium2 NeuronCores.

Sharding: core = (b, half) with b = core//2, half = core%2. Each core computes
output rows [half*1024, (half+1)*1024) of batch b: it builds K/V for the full
2048 keys of its batch (cheap projections) and Q only for its 1024 query rows,
so no cross-core reduction is needed — the host just concatenates row blocks.

Per-core pipeline (per head):
  S^T[m,n] = K^T.T @ Q^T on PE (float32r), exp on ACT (PSUM->SBUF, bf16 out),
  adjacency mask applied as a bf16 multiply on DVE, A@V as P^T.T @ [V|1] on PE
  (bf16) which yields both O and the softmax row-sums in one accumulation,
  normalization on DVE, PE-transpose of O into O^T for the output projection.
Softmax skips the row-max subtraction: scores are ~N(0,1) after the 1/sqrt(D)
scale (max |S| < ~7 over the whole problem), so exp stays in fp32/bf16 range.
node_mask only gates query rows (self-loops guarantee non-empty rows), so it
reduces to the final elementwise multiply.
"""

import math
from functools import lru_cache

import ml_dtypes
import numpy as np

import concourse.bacc as bacc
import concourse.mybir as mybir
import concourse.tile as tile
from concourse import masks

B, N, F = 4, 2048, 128
H, D = 8, 128
NQ = 1024  # query rows per core
NCORES = 8
EPS = 1e-5
SCALE = 1.0 / math.sqrt(D)

F32 = mybir.dt.float32
F32R = mybir.dt.float32r
BF16 = mybir.dt.bfloat16
AF = mybir.ActivationFunctionType
ALU = mybir.AluOpType

NMC = N // 128  # 16 m-chunks
NG = 2          # m-chunks per S/exp group (2 banks; double-buffered)
NCS = NQ // 512  # 2 n column chunks of 512


def _build_program(affine: bool = False):
    nc = bacc.Bacc(
        "TRN2", target_bir_lowering=False, debug=False, num_devices=NCORES
    )
    d_xT = nc.declare_dram_parameter("xT", [F, N], F32R, isOutput=False)
    d_xqT = nc.declare_dram_parameter("xqT", [F, NQ], F32R, isOutput=False)
    d_xres = nc.declare_dram_parameter("xres", [128, 8, 128], F32, isOutput=False)
    d_maskT = nc.declare_dram_parameter("maskT", [128, NMC, NQ], BF16, isOutput=False)
    d_wq = nc.declare_dram_parameter("wq", [F, H * D], F32R, isOutput=False)
    d_wk = nc.declare_dram_parameter("wk", [F, H * D], F32R, isOutput=False)
    d_wv = nc.declare_dram_parameter("wv", [F, H * D], F32R, isOutput=False)
    d_wo = nc.declare_dram_parameter("wo", [128, 8, 128], BF16, isOutput=False)
    d_gb = nc.declare_dram_parameter("gb", [2, 128], F32, isOutput=False)
    d_nm = nc.declare_dram_parameter("nm", [128, 8], F32, isOutput=False)
    d_out = nc.declare_dram_parameter("out", [128, 8, 128], F32, isOutput=True)

    with tile.TileContext(nc) as tc:
        with (
            tc.tile_pool(name="const", bufs=1) as const,
            tc.tile_pool(name="small", bufs=4) as sp,
        ):
            # order matters: the first head's projections need only x/w —
            # issue those first (x in slices) so PE starts while the 4MB
            # mask streams in behind them.
            wk = const.tile([128, H * D], F32R)
            nc.sync.dma_start(wk[:], d_wk[:])
            xT = const.tile([128, N], F32R)
            for j4 in range(4):
                nc.sync.dma_start(xT[:, j4 * 512:(j4 + 1) * 512],
                                    d_xT[:, j4 * 512:(j4 + 1) * 512])
            wq = const.tile([128, H * D], F32R)
            nc.sync.dma_start(wq[:], d_wq[:])
            xqT = const.tile([128, NQ], F32R)
            nc.sync.dma_start(xqT[:], d_xqT[:])
            wv = const.tile([128, H * D], F32R)
            nc.sync.dma_start(wv[:], d_wv[:])
            if affine:
                gbg = const.tile([1, 128], F32)
                nc.sync.dma_start(gbg[:], d_gb[0:1, :])
                gbb = const.tile([1, 128], F32)
                nc.sync.dma_start(gbb[:], d_gb[1:2, :])
            nm = const.tile([128, 8], F32)
            nc.sync.dma_start(nm[:], d_nm[:])
            maskT = const.tile([128, NMC * NQ], BF16)
            maskT_v = maskT[:].rearrange("p (c n) -> p c n", n=NQ)
            for mb4 in range(4):
                nc.sync.dma_start(maskT_v[:, mb4 * 4:(mb4 + 1) * 4, :],
                                    d_maskT[:, mb4 * 4:(mb4 + 1) * 4, :])
            wo = const.tile([128, 8 * 128], BF16)
            wo_v = wo[:].rearrange("p (h d) -> p h d", d=128)
            nc.sync.dma_start(wo_v, d_wo[:])
            xres = const.tile([128, 8 * 128], F32)
            xres_v = xres[:].rearrange("p (c d) -> p c d", d=128)
            nc.sync.dma_start(xres_v, d_xres[:])

            ident = const.tile([128, 128], BF16)
            masks.make_identity(nc, ident[:])
            ones1 = const.tile([1, 128], F32)
            nc.vector.memset(ones1[:], 1.0)
            eps_t = const.tile([128, 1], F32)
            nc.vector.memset(eps_t[:], EPS)

            OT = const.tile([128, H * NQ], BF16)  # O^T: [d, (h, n)]
            OT_v = OT[:].rearrange("p (h n) -> p h n", n=NQ)

            if affine:
                gamma_bc = const.tile([128, 128], F32)
                beta_bc = const.tile([128, 128], F32)

            with (
                tc.tile_pool(name="hp", bufs=3) as hp,
                tc.tile_pool(name="pp", bufs=4) as pp,
                tc.tile_pool(name="ps_p", bufs=1, space="PSUM") as ps_p,
                tc.tile_pool(name="ps_s", bufs=2, space="PSUM") as ps_s,
                tc.tile_pool(name="ps_av", bufs=1, space="PSUM") as ps_av,
                tc.tile_pool(name="ps_t", bufs=1, space="PSUM") as ps_t,
            ):
                for h in range(H):
                    hs = slice(h * 128, (h + 1) * 128)
                    # --- projections for this head ---
                    kt = hp.tile([128, N], F32R, tag="kt")
                    for j in range(4):
                        pj = ps_p.tile([128, 512], F32, tag="pj")
                        nc.tensor.matmul(
                            pj[:], wk[:, hs], xT[:, j * 512:(j + 1) * 512],
                            start=True, stop=True,
                        )
                        nc.vector.tensor_copy(kt[:, j * 512:(j + 1) * 512], pj[:])
                    qt = hp.tile([128, NQ], F32R, tag="qt")
                    for j in range(2):
                        pj = ps_p.tile([128, 512], F32, tag="pj")
                        nc.tensor.matmul(
                            pj[:], wq[:, hs], xqT[:, j * 512:(j + 1) * 512],
                            start=True, stop=True,
                        )
                        nc.scalar.copy(qt[:, j * 512:(j + 1) * 512], pj[:])
                    va = hp.tile([128, NMC * 130], BF16, tag="va")
                    va_v = va[:].rearrange("p (c k) -> p c k", k=130)
                    for j in range(4):
                        pj = ps_p.tile([128, 512], F32, tag="pj")
                        for c in range(4):
                            mc = j * 4 + c
                            nc.tensor.matmul(
                                pj[:, c * 128:(c + 1) * 128],
                                xT[:, mc * 128:(mc + 1) * 128], wv[:, hs],
                                start=True, stop=True,
                            )
                        nc.vector.tensor_copy(va_v[:, j * 4:(j + 1) * 4, 0:128],
                                              pj[:].rearrange("p (c d) -> p c d", d=128))
                    nc.vector.memset(va_v[:, :, 128:129], 1.0)

                    oh = hp.tile([128, 8 * 128], BF16, tag="oh")
                    oh_v = oh[:].rearrange("p (s d) -> p s d", d=128)

                    # --- attention over 512-wide query-column chunks ---
                    # Software-pipelined emission: PE executes in program
                    # order, so S(g+1) must be emitted BEFORE AV(g) (which
                    # waits on exp+mask of group g) or PE stalls every group.
                    NGRP = NMC // NG

                    def emit_s_group(g, qsl, kt=kt, qt=qt):
                        sg = ps_s.tile([128, NG * 512], F32, tag="sg", name="sg")
                        sg_v = sg[:].rearrange("p (c n) -> p c n", n=512)
                        for c in range(NG):
                            mc = g * NG + c
                            nc.tensor.matmul(
                                sg_v[:, c, :],
                                kt[:, mc * 128:(mc + 1) * 128], qt[:, qsl],
                                start=True, stop=True,
                            )
                        return sg_v

                    for ncs_i in range(NCS):
                        qsl = slice(ncs_i * 512, (ncs_i + 1) * 512)
                        avA = ps_av.tile([128, 512], F32, tag="avA")
                        avB = ps_av.tile([128, 512], F32, tag="avB")
                        sg_v = emit_s_group(0, qsl)
                        for g in range(NGRP):
                            praw = pp.tile([128, NG * 512], BF16, tag="praw")
                            praw_v = praw[:].rearrange("p (c n) -> p c n", n=512)
                            nc.scalar.activation(praw_v, sg_v, AF.Exp, scale=SCALE)
                            pt = pp.tile([128, NG * 512], BF16, tag="pt")
                            pt_v = pt[:].rearrange("p (c n) -> p c n", n=512)
                            nc.vector.tensor_tensor(
                                pt_v, praw_v,
                                maskT_v[:, g * NG:(g + 1) * NG, qsl], ALU.mult,
                            )
                            if g + 1 < NGRP:
                                sg_v = emit_s_group(g + 1, qsl)
                            for c in range(NG):
                                mc = g * NG + c
                                for s in range(4):
                                    av = avA if s < 2 else avB
                                    off = (s % 2) * 256
                                    # start/stop are bank-granular (the whole
                                    # 2KB zero-region is marked pending-zero),
                                    # so only the first/last matmul touching
                                    # each bank may carry them.
                                    nc.tensor.matmul(
                                        av[:, off:off + 129],
                                        pt_v[:, c, s * 128:(s + 1) * 128],
                                        va_v[:, mc, 0:129],
                                        start=(mc == 0 and s % 2 == 0),
                                        stop=(mc == NMC - 1 and s % 2 == 1),
                                    )
                        for t_i, av in ((0, avA), (1, avB)):
                            av_v = av[:].rearrange("p (r q) -> p r q", q=256)
                            rec2 = sp.tile([128, 2], F32, tag="rec")
                            nc.vector.reciprocal(rec2[:].unsqueeze(-1),
                                                 av_v[:, :, 128:129])
                            sp2 = slice(ncs_i * 4 + 2 * t_i,
                                        ncs_i * 4 + 2 * t_i + 2)
                            nc.vector.tensor_tensor(
                                oh_v[:, sp2, :], av_v[:, :, 0:128],
                                rec2[:].unsqueeze(-1).broadcast_to([128, 2, 128]),
                                ALU.mult)

                    # --- transpose O -> O^T for out-proj ---
                    for t2 in range(2):
                        tp = ps_t.tile([128, 512], BF16, tag="tp")
                        for s4 in range(4):
                            nc.tensor.matmul(
                                tp[:, s4 * 128:(s4 + 1) * 128],
                                oh_v[:, t2 * 4 + s4, :], ident[:],
                                is_transpose=True, start=True, stop=True,
                            )
                        nc.vector.tensor_copy(
                            OT_v[:, h, t2 * 512:(t2 + 1) * 512], tp[:]
                        )

            # --- out-proj + residual + LayerNorm + SiLU + node mask ---
            with (
                tc.tile_pool(name="fc", bufs=1) as fc,
                tc.tile_pool(name="ps_o", bufs=1, space="PSUM") as ps_o,
            ):
                if affine:
                    # broadcast gamma/beta across partitions via rank-1
                    # matmul (here, not at kernel start: PE is in-order, and
                    # waiting on the tiny gb DMA must not stall projections)
                    gps = ps_o.tile([128, 256], F32, tag="gps")
                    nc.tensor.matmul(gps[:, 0:128], ones1[:], gbg[:],
                                     start=True, stop=True)
                    nc.tensor.matmul(gps[:, 128:256], ones1[:], gbb[:],
                                     start=True, stop=True)
                    nc.vector.tensor_copy(gamma_bc[:], gps[:, 0:128])
                    nc.vector.tensor_copy(beta_bc[:], gps[:, 128:256])

                # out-proj for all 8 row chunks into one 2-bank PSUM span
                po_all = ps_o.tile([128, 8 * 128], F32, tag="po")
                for c in range(8):
                    for h2 in range(H):
                        nc.tensor.matmul(
                            po_all[:, c * 128:(c + 1) * 128],
                            OT_v[:, h2, c * 128:(c + 1) * 128], wo_v[:, h2, :],
                            start=(h2 == 0), stop=(h2 == H - 1),
                        )
                # residual + LayerNorm + SiLU + node mask, batched over 512-row
                # halves (two halves pipeline across DVE/ACT); per-(row,chunk)
                # stats applied via 0-stride broadcast APs.
                po_v = po_all[:].rearrange("p (c d) -> p c d", d=128)
                c3 = [128, 4, 128]
                for hf in range(2):
                    cs = slice(hf * 4, (hf + 1) * 4)
                    fo = fc.tile([128, 4 * 128], F32, tag="fo", bufs=2)
                    fo_v = fo[:].rearrange("p (c d) -> p c d", d=128)
                    nc.vector.tensor_tensor(fo_v, po_v[:, cs, :], xres_v[:, cs, :],
                                            ALU.add)
                    mu = fc.tile([128, 4], F32, tag="mu", bufs=2)
                    nc.vector.tensor_reduce(mu[:], fo_v, mybir.AxisListType.X,
                                            ALU.add)
                    mean = fc.tile([128, 4], F32, tag="mean", bufs=2)
                    nc.vector.tensor_scalar_mul(mean[:], mu[:], 1.0 / 128.0)
                    ctr = fc.tile([128, 4 * 128], F32, tag="ctr", bufs=2)
                    ctr_v = ctr[:].rearrange("p (c d) -> p c d", d=128)
                    nc.vector.tensor_tensor(
                        ctr_v, fo_v, mean[:].unsqueeze(-1).broadcast_to(c3),
                        ALU.subtract)
                    sq = fc.tile([128, 4 * 128], F32, tag="sq", bufs=2)
                    sq_v = sq[:].rearrange("p (c d) -> p c d", d=128)
                    nc.vector.tensor_tensor(sq_v, ctr_v, ctr_v, ALU.mult)
                    vs = fc.tile([128, 4], F32, tag="vs", bufs=2)
                    nc.vector.tensor_reduce(vs[:], sq_v, mybir.AxisListType.X,
                                            ALU.add)
                    # rsqrt(var+eps) = exp(-0.5*ln(var+eps)): both live in the
                    # natural_log_exp ACT set, far tighter splines than Sqrt
                    lnv = fc.tile([128, 4], F32, tag="lnv", bufs=2)
                    nc.scalar.activation(lnv[:], vs[:], AF.Ln, scale=1.0 / 128.0,
                                         bias=eps_t[:])
                    rs = fc.tile([128, 4], F32, tag="rs", bufs=2)
                    nc.scalar.activation(rs[:], lnv[:], AF.Exp, scale=-0.5)
                    nrm = fc.tile([128, 4 * 128], F32, tag="nrm", bufs=2)
                    nrm_v = nrm[:].rearrange("p (c d) -> p c d", d=128)
                    nc.vector.tensor_tensor(
                        nrm_v, ctr_v, rs[:].unsqueeze(-1).broadcast_to(c3),
                        ALU.mult)
                    if affine:
                        g1 = fc.tile([128, 4 * 128], F32, tag="g1", bufs=2)
                        g1_v = g1[:].rearrange("p (c d) -> p c d", d=128)
                        nc.vector.tensor_tensor(
                            g1_v, nrm_v,
                            gamma_bc[:].unsqueeze(1).broadcast_to(c3), ALU.mult)
                        g2 = fc.tile([128, 4 * 128], F32, tag="g2", bufs=2)
                        g2_v = g2[:].rearrange("p (c d) -> p c d", d=128)
                        nc.vector.tensor_tensor(
                            g2_v, g1_v,
                            beta_bc[:].unsqueeze(1).broadcast_to(c3), ALU.add)
                    else:
                        g2, g2_v = nrm, nrm_v
                    sig = fc.tile([128, 4 * 128], F32, tag="sig", bufs=2)
                    nc.scalar.activation(sig[:], g2[:], AF.Sigmoid)
                    sil = fc.tile([128, 4 * 128], F32, tag="sil", bufs=2)
                    sil_v = sil[:].rearrange("p (c d) -> p c d", d=128)
                    nc.vector.tensor_tensor(sil_v, g2_v, sig[:].rearrange(
                        "p (c d) -> p c d", d=128), ALU.mult)
                    fin = fc.tile([128, 4 * 128], F32, tag="fin", bufs=2)
                    fin_v = fin[:].rearrange("p (c d) -> p c d", d=128)
                    nc.vector.tensor_tensor(
                        fin_v, sil_v, nm[:, cs].unsqueeze(-1).broadcast_to(c3),
                        ALU.mult)
                    nc.sync.dma_start(d_out[:, cs, :], fin_v)

    nc.compile()
    return nc


@lru_cache(maxsize=2)
def _program(affine: bool = False):
    return _build_program(affine)


class _Executor:
    """Caches the jitted shard_map executable across kernel() calls (the
    fresh-jit path in run_bass_via_pjrt re-traces every call)."""

    def __init__(self, nc):
        import jax
        import concourse.mybir as mb
        from concourse import bass2jax
        from jax.sharding import Mesh, PartitionSpec
        from jax.experimental.shard_map import shard_map

        bass2jax.install_neuronx_cc_hook()
        self.jax = jax
        partition_name = (
            nc.partition_id_tensor.name if nc.partition_id_tensor else None
        )
        in_names, out_names, out_avals, zero_shapes = [], [], [], []
        for alloc in nc.m.functions[0].allocations:
            if not isinstance(alloc, mb.MemoryLocationSet):
                continue
            name = alloc.memorylocations[0].name
            if alloc.kind == "ExternalInput":
                if name != partition_name:
                    in_names.append(name)
            elif alloc.kind == "ExternalOutput":
                out_names.append(name)
                shape = tuple(alloc.tensor_shape)
                dtype = mb.dt.np(alloc.dtype)
                out_avals.append(jax.core.ShapedArray(shape, dtype))
                zero_shapes.append((shape, dtype))
        self.n_params = len(in_names)
        self.in_names = list(in_names)
        self.out_names = out_names
        self.out_avals = out_avals
        self.zero_shapes = zero_shapes
        all_in = in_names + out_names + ([partition_name] if partition_name else [])
        donate = tuple(range(self.n_params, self.n_params + len(out_names)))

        def _body(*args):
            operands = list(args)
            if partition_name is not None:
                operands.append(bass2jax.partition_id_tensor())
            return tuple(bass2jax._bass_exec_p.bind(
                *operands,
                out_avals=tuple(out_avals),
                in_names=tuple(all_in),
                out_names=tuple(out_names),
                lowering_input_output_aliases=(),
                sim_require_finite=True,
                sim_require_nnan=True,
                nc=nc,
            ))

        devices = jax.devices()[:NCORES]
        mesh = Mesh(np.asarray(devices), ("core",))
        n_in = self.n_params + len(out_names)
        self.sharded = jax.jit(
            shard_map(_body, mesh=mesh,
                      in_specs=(PartitionSpec("core"),) * n_in,
                      out_specs=(PartitionSpec("core"),) * len(out_names),
                      check_rep=False),
            donate_argnums=donate, keep_unused=True,
        )

    def concat_inputs(self, in_maps):
        return [
            np.concatenate([np.asarray(m[name]) for m in in_maps], axis=0)
            for name in self.in_names
        ]

    def zeros(self):
        return [np.zeros((NCORES * s[0], *s[1:]), d) for s, d in self.zero_shapes]

    def run(self, concat_in):
        out_arrs = self.sharded(*concat_in, *self.zeros())
        return out_arrs

    def split(self, out_arrs):
        return [
            {name: np.asarray(out_arrs[i]).reshape(NCORES, *self.out_avals[i].shape)[c]
             for i, name in enumerate(self.out_names)}
            for c in range(NCORES)
        ]


@lru_cache(maxsize=2)
def _executor(affine: bool = False):
    return _Executor(_program(affine))


def _prep_core_inputs(core, x, attn_mask, node_mask, Wq, Wk, Wv, Wo, bo,
                      gamma, beta):
    b, half = core // 2, core % 2
    rsl = slice(half * NQ, (half + 1) * NQ)
    xb = np.ascontiguousarray(x[b])
    m = {}
    m["xT"] = np.ascontiguousarray(xb.T)
    m["xqT"] = np.ascontiguousarray(xb[rsl].T)
    m["xres"] = np.ascontiguousarray(
        (xb[rsl] + bo).reshape(8, 128, 128).transpose(1, 0, 2)
    )
    mT = attn_mask[b].T[:, rsl].astype(np.float32)  # [2048 m, 1024 n]
    m["maskT"] = np.ascontiguousarray(
        mT.reshape(NMC, 128, NQ).transpose(1, 0, 2)
    ).astype(ml_dtypes.bfloat16)
    m["wq"], m["wk"], m["wv"] = Wq, Wk, Wv
    m["wo"] = np.ascontiguousarray(
        Wo.reshape(8, 128, 128).transpose(1, 0, 2)).astype(ml_dtypes.bfloat16)
    m["gb"] = np.ascontiguousarray(np.stack([gamma, beta]))
    m["nm"] = np.ascontiguousarray(
        node_mask[b, rsl].astype(np.float32).reshape(8, 128).T
    )
    return m


def kernel(x, attn_mask, node_mask, Wq, Wk, Wv, Wo, bo, gamma, beta):
    x = np.asarray(x, np.float32)
    attn_mask = np.asarray(attn_mask, bool)
    node_mask = np.asarray(node_mask, bool)
    Wq = np.ascontiguousarray(np.asarray(Wq, np.float32))
    Wk = np.ascontiguousarray(np.asarray(Wk, np.float32))
    Wv = np.ascontiguousarray(np.asarray(Wv, np.float32))
    Wo = np.asarray(Wo, np.float32)
    bo = np.asarray(bo, np.float32)
    gamma = np.asarray(gamma, np.float32)
    beta = np.asarray(beta, np.float32)

    affine = not (np.all(gamma == 1.0) and np.all(beta == 0.0))
    ex = _executor(affine)
    in_maps = [
        _prep_core_inputs(c, x, attn_mask, node_mask, Wq, Wk, Wv, Wo, bo,
                          gamma, beta)
        for c in range(NCORES)
    ]
    results = ex.split(ex.run(ex.concat_inputs(in_maps)))
    out = np.empty((B, N, D), np.float32)
    for core in range(NCORES):
        b, half = core // 2, core % 2
        o = results[core]["out"]  # [128, 8, 128]
        out[b, half * NQ:(half + 1) * NQ] = (
            o.transpose(1, 0, 2).reshape(NQ, 128)
        )
    return out
